# revision 19
# baseline (speedup 1.0000x reference)
"""Trainium2 Bass kernel: attention-LSTM decoder (nn_Attention_74698071212133).

Sharding: data-parallel over batch across 8 NeuronCores (64 rows each), weights
replicated.  Each core runs TWO mostly-independent 32-row recurrence streams
(chunks), phase-staggered half a step apart (tile_wait_until stamps steer the
Tile scheduler) so one stream's serial tail (softmax/ctx/gates/LSTM) hides
under the other stream's attention tanh chain (ScalarE tanh is the hard
per-step floor: B*T*H/8 elems / 128 lanes / 1.2GHz ~ 13.7us/step).

The h-recurrent + onehot gate matmuls and the output projection are fused
across the two streams (N=64, shared weight loads, 16-bit weights); the
ctx-dependent gate half stays per-stream (N=32, fp8 weights) to preserve the
stagger.  Precision: fp16 for all 16-bit tensors; fp8e3 only on the
LDWEIGHTS-bound paths (tanh scores, enc for ctxT, w_ctx), with scales folded
into activation scale parameters; sigmoid via 0.5*(tanh(x/2)+1) with h'=2h so
the 0.5s fold into host-scaled weights; scalar_tensor_tensor fuses (x+1)*y.
"""

import sys

sys.path.insert(0, "/opt/trn_rl_repo")

import numpy as np
import ml_dtypes

import concourse.bass as bass
import concourse.mybir as mybir
import concourse.tile as tile
from concourse import bacc
from concourse.bass_utils import run_bass_kernel_spmd

F16 = np.float16
F8 = ml_dtypes.float8_e3m4
F32 = mybir.dt.float32
FP16 = mybir.dt.float16
FP8 = mybir.dt.float8e3
AF = mybir.ActivationFunctionType
ALU = mybir.AluOpType

# Problem constants
B, T, D, H, C, S = 512, 64, 512, 512, 38, 26
NCORES = 8
BCORE = B // NCORES  # 64
NCHUNK = 2
G4 = 4 * H  # 2048
HK = H // 128  # 4 h-tiles
BC = BCORE // NCHUNK  # 32 batch per stream
BT = BC * T  # 2048 flattened (b, t) per stream
NJ = BT // 128  # 16

# fp8 path toggles
FP8_TH = True   # tanh output + w_score in fp8e3 (e-score LDW fast)
FP8_ENC = True  # enc for ctxT matmul in fp8e3
FP8_W = True    # ctx-gate weights in fp8e3 (x32 scaled)
WS_SC = 32.0 if FP8_TH else 1.0   # w_score host scale, folded out in exp
WG_SC = 32.0 if FP8_W else 1.0    # gate-weight host scale, folded out in ACT
ENC_SC = 4.0 if FP8_ENC else 1.0  # enc host scale, folded out in ctxT copy

TH_DT = FP8 if FP8_TH else FP16
ENC_DT = FP8 if FP8_ENC else FP16
W_DT = FP8 if FP8_W else FP16
TH_NP = F8 if FP8_TH else F16
ENC_NP = F8 if FP8_ENC else F16
W_NP = F8 if FP8_W else F16

# scheduler phase stagger (ms of model time per step)
STEP_MS = 0.021

# gate source order producing psum col layout [i | f | o | g]
GATE_ORDER = (0, 1, 3, 2)

# per-stream psum scratch-bank column layout (f32 cols of a [128, 512] tile)
PE20, PE21 = 0, NJ                    # e-scores [128, 16]
PTR0, PTR1 = 16, 144                  # e transposed [16, 128]
PAC0, PAC1 = 144, 160                 # alpha transposed back [128, 16]
PCTX0, PCTX1 = 160, 288               # ctxT [128, 128]
PP0, PP1 = 288, 326                   # probs [64, 38] (stream-0 bank only)
PHP0, PHP1 = 326, 454                 # hp for next step [128, 128]


def _tile128(a, dt):
    """[R, N] with R = r*128 -> [128, r*N] col-block layout (block k = rows 128k..)."""
    r = a.shape[0] // 128
    return np.ascontiguousarray(
        a.reshape(r, 128, a.shape[1]).transpose(1, 0, 2).reshape(128, -1)
    ).astype(dt)


class Ctx:
    """Per-build handles."""


def build_nc(steps=S):
    nc = bacc.Bacc()
    dp = nc.declare_dram_parameter
    x = Ctx()
    x.nc = nc
    x.steps = steps

    d_enc = dp("enc_sb", [NCHUNK, 128, NJ * 512], ENC_DT, isOutput=False)
    d_encT = dp("encT_sb", [NCHUNK, 128, HK * BT], FP16, isOutput=False)
    d_oh = dp("ohT_sb", [128, steps * BCORE], FP16, isOutput=False)
    d_wi2h = dp("w_i2hT", [128, HK * H], FP16, isOutput=False)
    d_wh2h = dp("w_h2hT", [128, HK * H], FP16, isOutput=False)
    d_wsc = dp("w_scoreT", [128, HK], TH_DT, isOutput=False)
    d_wctx = dp("w_ctxT", [128, HK * G4], W_DT, isOutput=False)
    d_whh = dp("w_hhT", [128, HK * G4], FP16, isOutput=False)
    d_woh = dp("w_ohT", [128, G4], FP16, isOutput=False)
    d_wgen = dp("w_genT", [128, HK * C], FP16, isOutput=False)
    d_bgen = dp("b_gen", [1, C], FP16, isOutput=False)
    d_bh2h = dp("b_h2hT", [128, HK], F32, isOutput=False)
    d_idf = dp("id_f", [128, 128], F32, isOutput=False)
    d_ones = dp("ones_row", [1, BCORE], FP16, isOutput=False)
    d_out = dp("probs", [BCORE, steps, C], F32, isOutput=True)

    with tile.TileContext(nc) as tc:
        with (
            tc.tile_pool(name="consts", bufs=1) as pc,
            tc.tile_pool(name="persist", bufs=1) as pp,
        ):
            def cload(dram, shape, dt):
                t_ = pc.tile(list(shape), dt, name=dram.tensor.name + "_sb")
                nc.sync.dma_start(t_[:], dram)
                return t_

            x.w_h2h = cload(d_wh2h[:], [128, HK * H], FP16)
            x.w_sc = cload(d_wsc[:], [128, HK], TH_DT)
            x.w_ctx = cload(d_wctx[:], [128, HK * G4], W_DT)
            x.w_hh = cload(d_whh[:], [128, HK * G4], FP16)
            x.w_oh = cload(d_woh[:], [128, G4], FP16)
            x.w_gen = cload(d_wgen[:], [128, HK * C], FP16)
            x.b_gen = cload(d_bgen[:], [1, C], FP16)
            x.id_f = cload(d_idf[:], [128, 128], F32)
            x.ones = cload(d_ones[:], [1, BCORE], FP16)
            x.ohT = cload(d_oh[:], [128, steps * BCORE], FP16)
            w_i2h = cload(d_wi2h[:], [128, HK * H], FP16)
            b_h2h = cload(d_bh2h[:], [128, HK], F32)

            # ---- persistent state: hT fused across streams, rest per-stream ----
            x.hTf = pp.tile([128, HK * BCORE], FP16, tag="hTf", name="hTf")
            nc.vector.memset(x.hTf[:], 0.0)
            x.cT, x.ctxT, x.ad, x.enc_sb, x.hproj = [], [], [], [], []
            for c in range(NCHUNK):
                c_ = pp.tile([128, HK * BC], F32, tag=f"cT{c}", name=f"cT{c}")
                ct = pp.tile([128, HK * BC], FP16, tag=f"ctxT{c}", name=f"ctxT{c}")
                a_ = pp.tile([128, 2 * NJ], FP16, tag=f"ad{c}", name=f"ad{c}")
                nc.vector.memset(a_[:], 0.0)
                x.cT.append(c_)
                x.ctxT.append(ct)
                x.ad.append(a_)
                e_ = pp.tile([128, NJ * 512], ENC_DT, tag=f"enc{c}", name=f"enc{c}")
                for q in range(4):
                    w = NJ * 512 // 4
                    nc.sync.dma_start(
                        e_[:, q * w : (q + 1) * w], d_enc[c, :, q * w : (q + 1) * w]
                    )
                x.enc_sb.append(e_)
                x.hproj.append(
                    pp.tile([128, HK * BT], FP16, tag=f"hproj{c}", name=f"hproj{c}")
                )

            # ---- init: H_projT = w_i2h @ encT + b_h2h ----
            with (
                tc.tile_pool(name="encT", bufs=1) as pet,
                tc.tile_pool(name="initps", bufs=4, space="PSUM") as pips,
            ):
                for c in range(NCHUNK):
                    et = pet.tile([128, HK * BT], FP16, tag=f"encT{c}", name=f"encT{c}")
                    for q in range(4):
                        w = HK * BT // 4
                        nc.sync.dma_start(
                            et[:, q * w : (q + 1) * w],
                            d_encT[c, :, q * w : (q + 1) * w],
                        )
                    for m in range(HK):
                        for n in range(BT // 512):
                            ps = pips.tile([128, 512], F32, tag="initp")
                            for k in range(HK):
                                nc.tensor.matmul(
                                    ps[:],
                                    w_i2h[:, k * H + 128 * m : k * H + 128 * m + 128],
                                    et[:, k * BT + 512 * n : k * BT + 512 * n + 512],
                                    start=(k == 0),
                                    stop=(k == HK - 1),
                                )
                            nc.scalar.activation(
                                x.hproj[c][:, m * BT + 512 * n : m * BT + 512 * n + 512],
                                ps[:],
                                AF.Identity,
                                bias=b_h2h[:, m : m + 1],
                            )

            # ---- decode: phase-staggered two-stream pipeline ----
            with (
                tc.tile_pool(name="work", bufs=1) as pw,
                tc.tile_pool(name="small", bufs=1) as psm,
                tc.tile_pool(name="ps", bufs=1, space="PSUM") as ps,
            ):
                x.pw, x.psm, x.ps = pw, psm, ps
                x.d_out = d_out
                x.pg = None
                x.sc = [
                    ps.tile([128, 512], F32, tag=f"sc{c}", name=f"sc{c}", bufs=1)
                    for c in range(NCHUNK)
                ]
                x.th = [[None, None], [None, None]]

                def ph(w):
                    return tc.tile_wait_until(w * STEP_MS)

                with ph(0):
                    gates_hof(x, 0)
                with ph(0):
                    p1a(x, 0, 0)
                    p1b(x, 0, 0)
                with ph(0.5):
                    p1a(x, 1, 0)
                    p1b(x, 1, 0)
                for s in range(steps):
                    with ph(s + 0.35):
                        p2(x, 0, s)
                    with ph(s + 0.85):
                        p2(x, 1, s)
                        probs_f(x, s)
                        if s + 1 < steps:
                            gates_hof(x, s + 1)
                    if s + 1 < steps:
                        with ph(s + 1.0):
                            p1a(x, 0, s + 1)
                            p1b(x, 0, s + 1)
                        with ph(s + 1.5):
                            p1a(x, 1, s + 1)
                            p1b(x, 1, s + 1)
    if not nc.is_finalized():
        nc.finalize()
    return nc


def gates_hof(x, s):
    """Fused h-recurrent + onehot gate matmuls for BOTH streams (N=64).
    pg col layout [gi(4), k(4), b(64)] = [i | f | o | g], banks A=(i,f) B=(o,g)."""
    nc = x.nc
    pg = x.ps.tile([128, 1024], F32, tag="pg", name=f"pg{s}", bufs=2)
    x.pg = pg
    ohsl = x.ohT[:, s * BCORE : (s + 1) * BCORE]
    started = set()
    for gi, g in enumerate(GATE_ORDER):
        for k in range(HK):
            m = 4 * g + k
            col = pg[:, gi * 256 + k * BCORE : gi * 256 + (k + 1) * BCORE]
            bank = gi // 2
            for kk in range(HK):
                nc.tensor.matmul(
                    col,
                    x.w_hh[:, kk * G4 + 128 * m : kk * G4 + 128 * m + 128],
                    x.hTf[:, kk * BCORE : (kk + 1) * BCORE],
                    start=(bank not in started),
                    stop=False,
                    skip_group_check=True,
                )
                started.add(bank)
            nc.tensor.matmul(
                col, x.w_oh[:, 128 * m : 128 * m + 128], ohsl,
                start=False, stop=False, skip_group_check=True,
            )


def probs_f(x, s):
    """Fused output projection for both streams: [64, C] psum -> DRAM."""
    nc = x.nc
    pp_ = x.sc[0][0:BCORE, PP0:PP1]
    for k in range(HK):
        nc.tensor.matmul(
            pp_,
            x.hTf[:, k * BCORE : (k + 1) * BCORE],
            x.w_gen[:, k * C : (k + 1) * C],
            start=(k == 0),
            stop=False,
            skip_group_check=True,
        )
    nc.tensor.matmul(
        pp_, x.ones[0:1, :], x.b_gen[:],
        start=False, stop=True, skip_group_check=True,
    )
    po = x.psm.tile([BCORE, C], F32, tag="po", name="po", bufs=2)
    nc.vector.tensor_copy(po[:], pp_)
    nc.sync.dma_start(x.d_out[:, s, :], po[:])


def p1a(x, c, s):
    """Attention front half (DVE+ACT): hp broadcast, add, tanh -> th tiles."""
    nc = x.nc
    if s > 0:
        hp2 = x.psm.tile([128, HK * BC * 2], FP16, tag=f"hp2_{c}", name=f"hp2_{c}",
                         bufs=2)
        nc.vector.tensor_copy(
            hp2[:].rearrange("p (k b two) -> p k b two", k=HK, two=2),
            x.sc[c][:, PHP0:PHP1]
            .rearrange("p (k b) -> p k b", k=HK)
            .unsqueeze(3)
            .broadcast_to([128, HK, BC, 2]),
        )
        hp2v = hp2[:].rearrange("p (k b two) -> p k b two", k=HK, two=2)
    for kp in range(2):
        th = x.pw.tile([128, 2 * BT], TH_DT, tag=f"th{c}", name=f"th{c}_{kp}", bufs=2)
        x.th[c][kp] = th
        if s == 0:
            nc.scalar.activation(
                th[:], x.hproj[c][:, kp * 2 * BT : (kp + 1) * 2 * BT], AF.Tanh
            )
        else:
            a = x.pw.tile([128, 2 * BT], FP16, tag=f"a{c}", name=f"a{c}_{kp}", bufs=2)
            for kk in range(2):
                k = 2 * kp + kk
                nc.vector.tensor_add(
                    a[:, kk * BT : (kk + 1) * BT].rearrange(
                        "p (b t2 two) -> p b t2 two", b=BC, two=2
                    ),
                    x.hproj[c][:, k * BT : (k + 1) * BT].rearrange(
                        "p (b t2 two) -> p b t2 two", b=BC, two=2
                    ),
                    hp2v[:, k].unsqueeze(2).broadcast_to([128, BC, T // 2, 2]),
                )
            nc.scalar.activation(th[:], a[:], AF.Tanh)


def p1b(x, c, s):
    """Attention e-scores (PE): pe2[:, j] = sum_k th_k[:,128j:].T @ w_sc[:,k]."""
    nc = x.nc
    pe2 = x.sc[c][:, PE20:PE21]
    for kp in range(2):
        th = x.th[c][kp]
        for kk in range(2):
            k = 2 * kp + kk
            for j in range(NJ):
                nc.tensor.matmul(
                    pe2[:, j : j + 1],
                    th[:, kk * BT + 128 * j : kk * BT + 128 * j + 128],
                    x.w_sc[:, k : k + 1],
                    start=(k == 0 and j == 0),
                    stop=(k == HK - 1 and j == NJ - 1),
                    skip_group_check=True,
                )


def p2(x, c, s):
    """Stream tail: softmax, ctxT, ctx-gates, LSTM, php."""
    nc = x.nc
    steps = x.steps

    # -- softmax over t (transpose dance; no max-sub: |e| small) --
    e2 = x.psm.tile([128, NJ], F32, tag=f"e2_{c}", name=f"e2_{c}", bufs=2)
    nc.vector.tensor_copy(e2[:], x.sc[c][:, PE20:PE21])
    ptr = x.sc[c][0:NJ, PTR0:PTR1]
    nc.tensor.transpose(ptr, e2[:], x.id_f[:])
    ex = x.psm.tile([NJ, 128], FP16, tag=f"ex_{c}", name=f"ex_{c}", bufs=2)
    nc.scalar.activation(ex[:], ptr, AF.Exp, scale=1.0 / WS_SC)
    ssum = x.psm.tile([NJ, 2], F32, tag=f"ssum_{c}", name=f"ssum_{c}", bufs=2)
    nc.vector.reduce_sum(
        ssum[:], ex[:].rearrange("p (b t) -> p b t", b=2), axis=mybir.AxisListType.X
    )
    rinv = x.psm.tile([NJ, 2], F32, tag=f"rinv_{c}", name=f"rinv_{c}", bufs=2)
    nc.vector.reciprocal(rinv[:], ssum[:])
    al = x.psm.tile([NJ, 128], F32, tag=f"al_{c}", name=f"al_{c}", bufs=2)
    nc.vector.tensor_mul(
        al[:].rearrange("p (b t) -> p b t", b=2),
        ex[:].rearrange("p (b t) -> p b t", b=2),
        rinv[:].unsqueeze(2).broadcast_to([NJ, 2, T]),
    )
    pac = x.sc[c][:, PAC0:PAC1]
    nc.tensor.transpose(pac, al[:], x.id_f[0:NJ, 0:NJ])
    adv = x.ad[c][:].rearrange("p (i two) -> p i two", two=2)
    for jj in range(2):
        nc.vector.tensor_copy(
            adv[64 * jj : 64 * jj + 64, :, jj], pac[64 * jj : 64 * jj + 64, :]
        )

    # -- ctxT[d, b] direct: lhsT = enc tile (d -> partitions), rhs = ad pair --
    pctxT = x.sc[c][:, PCTX0:PCTX1]
    for m in range(HK):
        for i in range(BC // 2):
            nc.tensor.matmul(
                pctxT[:, m * BC + 2 * i : m * BC + 2 * i + 2],
                x.enc_sb[c][:, 512 * i + 128 * m : 512 * i + 128 * m + 128],
                x.ad[c][:, 2 * i : 2 * i + 2],
                start=True,
                stop=True,
            )
    nc.vector.tensor_scalar_mul(x.ctxT[c][:], pctxT, 1.0 / ENC_SC)

    # -- ctx contribution to gates (per-stream half, N=32) --
    pg = x.pg
    for gi, g in enumerate(GATE_ORDER):
        for k in range(HK):
            m = 4 * g + k
            col = pg[:, gi * 256 + k * BCORE + c * BC : gi * 256 + k * BCORE + (c + 1) * BC]
            for kk in range(HK):
                nc.tensor.matmul(
                    col,
                    x.w_ctx[:, kk * G4 + 128 * m : kk * G4 + 128 * m + 128],
                    x.ctxT[c][:, kk * BC : (kk + 1) * BC],
                    start=False,
                    stop=(c == 1 and gi % 2 == 1 and k == HK - 1 and kk == HK - 1),
                    skip_group_check=True,
                )

    # -- LSTM (h' = 2h trick: sigma(x) = 0.5*(tanh(x/2)+1)) --
    pgv = pg[:].rearrange("p (gi k b) -> p gi k b", gi=4, k=HK)
    tifo = x.psm.tile([128, 3 * HK * BC], FP16, tag=f"tifo_{c}", name=f"tifo_{c}",
                      bufs=2)
    nc.scalar.activation(
        tifo[:].rearrange("p (gi k b) -> p gi k b", gi=3, k=HK),
        pgv[:, 0:3, :, c * BC : (c + 1) * BC],
        AF.Tanh, scale=0.5 / WG_SC,
    )
    tg = x.psm.tile([128, HK * BC], FP16, tag=f"tg_{c}", name=f"tg_{c}", bufs=2)
    nc.scalar.activation(
        tg[:].rearrange("p (k b) -> p k b", k=HK),
        pgv[:, 3, :, c * BC : (c + 1) * BC],
        AF.Tanh, scale=1.0 / WG_SC,
    )
    # C' = 2c: C'_new = 0.5*(tanh_f+1)*C'_old + (tanh_i+1)*tanh_g
    m2 = x.psm.tile([128, HK * BC], F32, tag=f"m2_{c}", name=f"m2_{c}", bufs=2)
    nc.vector.scalar_tensor_tensor(
        m2[:], tifo[:, 0 : HK * BC], 1.0, tg[:], ALU.add, ALU.mult
    )
    if s == 0:
        nc.vector.tensor_copy(x.cT[c][:], m2[:])
    else:
        m1 = x.psm.tile([128, HK * BC], F32, tag=f"m1_{c}", name=f"m1_{c}", bufs=2)
        nc.vector.scalar_tensor_tensor(
            m1[:], tifo[:, HK * BC : 2 * HK * BC], 1.0, x.cT[c][:], ALU.add, ALU.mult
        )
        nc.vector.scalar_tensor_tensor(
            x.cT[c][:], m1[:], 0.5, m2[:], ALU.mult, ALU.add
        )
    tc_ = x.psm.tile([128, HK * BC], FP16, tag=f"tc_{c}", name=f"tc_{c}", bufs=2)
    nc.scalar.activation(tc_[:], x.cT[c][:], AF.Tanh, scale=0.5)
    # H' = 2h = (tanh_o + 1) * tanh(c), into this stream's half of fused hT
    nc.vector.scalar_tensor_tensor(
        x.hTf[:].rearrange("p (k b) -> p k b", k=HK)[:, :, c * BC : (c + 1) * BC],
        tifo[:].rearrange("p (gi k b) -> p gi k b", gi=3, k=HK)[:, 2],
        1.0,
        tc_[:].rearrange("p (k b) -> p k b", k=HK),
        ALU.add, ALU.mult,
    )

    # -- php = w_h2h' @ H' (next step's attention shift) --
    if s < steps - 1:
        php = x.sc[c][:, PHP0:PHP1]
        for m in range(HK):
            for k in range(HK):
                nc.tensor.matmul(
                    php[:, m * BC : (m + 1) * BC],
                    x.w_h2h[:, k * H + 128 * m : k * H + 128 * m + 128],
                    x.hTf[:, k * BCORE + c * BC : k * BCORE + (c + 1) * BC],
                    start=(m == 0 and k == 0),
                    stop=(m == HK - 1 and k == HK - 1),
                    skip_group_check=True,
                )


# ------------------------- host side -------------------------


def prep_inputs(encoder_output, text, w_i2h, w_h2h, b_h2h, w_score, w_ih, w_hh,
                b_ih, b_hh, w_gen, b_gen, steps=S):
    """Build per-core input maps (numpy only)."""
    enc = np.asarray(encoder_output, np.float32)
    text = np.asarray(text)

    wid = {}
    wid["w_i2hT"] = _tile128(np.asarray(w_i2h, np.float32).T, F16)
    # w_h2h' = 0.5*w_h2h: php(H') = w_h2h @ h exactly
    wid["w_h2hT"] = _tile128(0.5 * np.asarray(w_h2h, np.float32).T, F16)
    wid["w_scoreT"] = _tile128(
        WS_SC * np.asarray(w_score, np.float32).reshape(H, 1), TH_NP
    )
    # gate weights: ctx part xWG, h part xWG/2 (h'=2h), oh part + bias xWG
    wid["w_ctxT"] = _tile128(WG_SC * np.asarray(w_ih, np.float32)[:, :D].T, W_NP)
    wid["w_hhT"] = _tile128(0.5 * WG_SC * np.asarray(w_hh, np.float32).T, F16)
    woh = np.zeros((128, G4), np.float32)
    woh[:C] = WG_SC * np.asarray(w_ih, np.float32)[:, D:].T
    woh[C] = WG_SC * (np.asarray(b_ih, np.float32) + np.asarray(b_hh, np.float32))
    wid["w_ohT"] = woh.astype(F16)
    wid["w_genT"] = _tile128(0.5 * np.asarray(w_gen, np.float32).T, F16)
    wid["b_gen"] = np.asarray(b_gen, np.float32).reshape(1, C).astype(F16)
    wid["b_h2hT"] = np.ascontiguousarray(
        np.asarray(b_h2h, np.float32).reshape(HK, 128).T
    )
    wid["id_f"] = np.eye(128, dtype=np.float32)
    wid["ones_row"] = np.ones((1, BCORE), F16)

    in_maps = []
    for core in range(NCORES):
        rows = slice(core * BCORE, (core + 1) * BCORE)
        ec = enc[rows]  # [64, T, D]
        enc_sb = np.zeros((NCHUNK, 128, NJ * 512), ENC_NP)
        encT_sb = np.zeros((NCHUNK, 128, HK * BT), F16)
        for c in range(NCHUNK):
            flat = ec[c * BC : (c + 1) * BC].reshape(BT, D)  # b-major (b,t) rows
            enc_sb[c] = _tile128(np.clip(ENC_SC * flat, -15.5, 15.5), ENC_NP)
            encT_sb[c] = _tile128(np.ascontiguousarray(flat.T), F16)
        oh = np.zeros((128, steps * BCORE), F16)
        tx = text[rows]  # [64, S]
        for s in range(steps):
            oh[tx[:, s].astype(np.int64), s * BCORE + np.arange(BCORE)] = 1.0
        oh[C] = 1.0
        m = dict(wid)
        m["enc_sb"] = enc_sb
        m["encT_sb"] = encT_sb
        m["ohT_sb"] = oh
        in_maps.append(m)
    return in_maps


_NC_CACHE = {}


def get_nc(steps=S):
    if steps not in _NC_CACHE:
        _NC_CACHE[steps] = build_nc(steps)
    return _NC_CACHE[steps]


def run(inputs, steps=S, trace=False):
    nc = get_nc(steps)
    in_maps = prep_inputs(**inputs, steps=steps)
    res = run_bass_kernel_spmd(nc, in_maps, list(range(NCORES)), trace=trace)
    out = np.concatenate([res.results[i]["probs"] for i in range(NCORES)], axis=0)
    return out.astype(np.float32), res


def kernel(**inputs):
    out, _ = run(inputs)
    return out


# revision 20
# speedup vs baseline: 1.1295x; 1.1295x over previous
"""Trainium2 Bass kernel: attention-LSTM decoder (nn_Attention_74698071212133).

Sharding: data-parallel over batch across 8 NeuronCores (64 rows each), weights
replicated.  Each core runs TWO mostly-independent 32-row recurrence streams
(chunks), phase-staggered half a step apart (tile_wait_until stamps steer the
Tile scheduler) so one stream's serial tail (softmax/ctx/gates/LSTM) hides
under the other stream's attention tanh chain (ScalarE tanh is the hard
per-step floor: B*T*H/8 elems / 128 lanes / 1.2GHz ~ 13.7us/step).

The h-recurrent + onehot gate matmuls and the output projection are fused
across the two streams (N=64, shared weight loads, 16-bit weights); the
ctx-dependent gate half stays per-stream (N=32, fp8 weights) to preserve the
stagger.  Precision: fp16 for all 16-bit tensors; fp8e3 only on the
LDWEIGHTS-bound paths (tanh scores, enc for ctxT, w_ctx), with scales folded
into activation scale parameters; sigmoid via 0.5*(tanh(x/2)+1) with h'=2h so
the 0.5s fold into host-scaled weights; scalar_tensor_tensor fuses (x+1)*y.
"""

import sys

sys.path.insert(0, "/opt/trn_rl_repo")

import numpy as np
import ml_dtypes

import concourse.bass as bass
import concourse.mybir as mybir
import concourse.tile as tile
from concourse import bacc
from concourse.bass_utils import run_bass_kernel_spmd

F16 = np.float16
F8 = ml_dtypes.float8_e3m4
F32 = mybir.dt.float32
FP16 = mybir.dt.float16
FP8 = mybir.dt.float8e3
AF = mybir.ActivationFunctionType
ALU = mybir.AluOpType

# Problem constants
B, T, D, H, C, S = 512, 64, 512, 512, 38, 26
NCORES = 8
BCORE = B // NCORES  # 64
NCHUNK = 2
G4 = 4 * H  # 2048
HK = H // 128  # 4 h-tiles
BC = BCORE // NCHUNK  # 32 batch per stream
BT = BC * T  # 2048 flattened (b, t) per stream
NJ = BT // 128  # 16

# fp8 path toggles
FP8_TH = True   # tanh output + w_score in fp8e3 (e-score LDW fast)
FP8_ENC = True  # enc for ctxT matmul in fp8e3
FP8_W = True    # ctx-gate weights in fp8e3 (x32 scaled)
WS_SC = 32.0 if FP8_TH else 1.0   # w_score host scale, folded out in exp
WG_SC = 32.0 if FP8_W else 1.0    # gate-weight host scale, folded out in ACT
ENC_SC = 4.0 if FP8_ENC else 1.0  # enc host scale, folded out in ctxT copy

TH_DT = FP8 if FP8_TH else FP16
ENC_DT = FP8 if FP8_ENC else FP16
W_DT = FP8 if FP8_W else FP16
TH_NP = F8 if FP8_TH else F16
ENC_NP = F8 if FP8_ENC else F16
W_NP = F8 if FP8_W else F16

# scheduler phase stagger (ms of model time per step)
STEP_MS = 0.021

# gate source order producing psum col layout [i | f | o | g]
GATE_ORDER = (0, 1, 3, 2)

# per-stream psum scratch-bank column layout (f32 cols of a [128, 512] tile)
PE20, PE21 = 0, NJ                    # e-scores [128, 16]
PTR0, PTR1 = 16, 144                  # e transposed [16, 128]
PAC0, PAC1 = 144, 160                 # alpha transposed back [128, 16]
PCTX0, PCTX1 = 160, 288               # ctxT [128, 128]
PP0, PP1 = 288, 326                   # probs [64, 38] (stream-0 bank only)
PHP0, PHP1 = 326, 454                 # hp for next step [128, 128]


def _tile128(a, dt):
    """[R, N] with R = r*128 -> [128, r*N] col-block layout (block k = rows 128k..)."""
    r = a.shape[0] // 128
    return np.ascontiguousarray(
        a.reshape(r, 128, a.shape[1]).transpose(1, 0, 2).reshape(128, -1)
    ).astype(dt)


class Ctx:
    """Per-build handles."""


def build_nc(steps=S):
    nc = bacc.Bacc()
    dp = nc.declare_dram_parameter
    x = Ctx()
    x.nc = nc
    x.steps = steps

    d_enc = dp("enc_sb", [NCHUNK, 128, NJ * 512], ENC_DT, isOutput=False)
    d_encT = dp("encT_sb", [NCHUNK, 128, HK * BT], FP16, isOutput=False)
    d_oh = dp("ohT_sb", [128, steps * BCORE], FP16, isOutput=False)
    d_wi2h = dp("w_i2hT", [128, HK * H], FP16, isOutput=False)
    d_wh2h = dp("w_h2hT", [128, HK * H], FP16, isOutput=False)
    d_wsc = dp("w_scoreT", [128, HK], TH_DT, isOutput=False)
    d_wctx = dp("w_ctxT", [128, HK * G4], W_DT, isOutput=False)
    d_whh = dp("w_hhT", [128, HK * G4], FP16, isOutput=False)
    d_woh = dp("w_ohT", [128, G4], FP16, isOutput=False)
    d_wgen = dp("w_genT", [128, HK * C], FP16, isOutput=False)
    d_bgen = dp("b_gen", [1, C], FP16, isOutput=False)
    d_bh2h = dp("b_h2hT", [128, HK], F32, isOutput=False)
    d_idf = dp("id_f", [128, 128], F32, isOutput=False)
    d_ones = dp("ones_row", [1, BCORE], FP16, isOutput=False)
    d_out = dp("probs", [BCORE, steps, C], F32, isOutput=True)

    with tile.TileContext(nc) as tc:
        with (
            tc.tile_pool(name="consts", bufs=1) as pc,
            tc.tile_pool(name="persist", bufs=1) as pp,
        ):
            def cload(dram, shape, dt):
                t_ = pc.tile(list(shape), dt, name=dram.tensor.name + "_sb")
                nc.sync.dma_start(t_[:], dram)
                return t_

            x.w_h2h = cload(d_wh2h[:], [128, HK * H], FP16)
            x.w_sc = cload(d_wsc[:], [128, HK], TH_DT)
            x.w_ctx = cload(d_wctx[:], [128, HK * G4], W_DT)
            x.w_hh = cload(d_whh[:], [128, HK * G4], FP16)
            x.w_oh = cload(d_woh[:], [128, G4], FP16)
            x.w_gen = cload(d_wgen[:], [128, HK * C], FP16)
            x.b_gen = cload(d_bgen[:], [1, C], FP16)
            x.id_f = cload(d_idf[:], [128, 128], F32)
            x.ones = cload(d_ones[:], [1, BCORE], FP16)
            x.ohT = cload(d_oh[:], [128, steps * BCORE], FP16)
            w_i2h = cload(d_wi2h[:], [128, HK * H], FP16)
            b_h2h = cload(d_bh2h[:], [128, HK], F32)

            # ---- persistent state: hT fused across streams, rest per-stream ----
            x.hTf = pp.tile([128, HK * BCORE], FP16, tag="hTf", name="hTf")
            nc.vector.memset(x.hTf[:], 0.0)
            x.cT, x.ctxT, x.ad, x.enc_sb, x.hproj = [], [], [], [], []
            for c in range(NCHUNK):
                c_ = pp.tile([128, HK * BC], F32, tag=f"cT{c}", name=f"cT{c}")
                ct = pp.tile([128, HK * BC], FP16, tag=f"ctxT{c}", name=f"ctxT{c}")
                a_ = pp.tile([128, 2 * NJ], FP16, tag=f"ad{c}", name=f"ad{c}")
                nc.vector.memset(a_[:], 0.0)
                x.cT.append(c_)
                x.ctxT.append(ct)
                x.ad.append(a_)
                e_ = pp.tile([128, NJ * 512], ENC_DT, tag=f"enc{c}", name=f"enc{c}")
                for q in range(4):
                    w = NJ * 512 // 4
                    nc.sync.dma_start(
                        e_[:, q * w : (q + 1) * w], d_enc[c, :, q * w : (q + 1) * w]
                    )
                x.enc_sb.append(e_)
                x.hproj.append(
                    pp.tile([128, HK * BT], FP16, tag=f"hproj{c}", name=f"hproj{c}")
                )

            # ---- init: H_projT = w_i2h @ encT + b_h2h ----
            with (
                tc.tile_pool(name="encT", bufs=1) as pet,
                tc.tile_pool(name="initps", bufs=4, space="PSUM") as pips,
            ):
                for c in range(NCHUNK):
                    et = pet.tile([128, HK * BT], FP16, tag=f"encT{c}", name=f"encT{c}")
                    for q in range(4):
                        w = HK * BT // 4
                        nc.sync.dma_start(
                            et[:, q * w : (q + 1) * w],
                            d_encT[c, :, q * w : (q + 1) * w],
                        )
                    for m in range(HK):
                        for n in range(BT // 512):
                            ps = pips.tile([128, 512], F32, tag="initp")
                            for k in range(HK):
                                nc.tensor.matmul(
                                    ps[:],
                                    w_i2h[:, k * H + 128 * m : k * H + 128 * m + 128],
                                    et[:, k * BT + 512 * n : k * BT + 512 * n + 512],
                                    start=(k == 0),
                                    stop=(k == HK - 1),
                                )
                            nc.scalar.activation(
                                x.hproj[c][:, m * BT + 512 * n : m * BT + 512 * n + 512],
                                ps[:],
                                AF.Identity,
                                bias=b_h2h[:, m : m + 1],
                            )

            # ---- decode: phase-staggered two-stream pipeline ----
            with (
                tc.tile_pool(name="work", bufs=1) as pw,
                tc.tile_pool(name="small", bufs=1) as psm,
                tc.tile_pool(name="ps", bufs=1, space="PSUM") as ps,
            ):
                x.pw, x.psm, x.ps = pw, psm, ps
                x.d_out = d_out
                x.pg = None
                x.sc = [
                    ps.tile([128, 512], F32, tag=f"sc{c}", name=f"sc{c}", bufs=1)
                    for c in range(NCHUNK)
                ]
                x.th = [[None, None], [None, None]]

                gates_hof(x, 0)
                p1a(x, 0, 0)
                p1b(x, 0, 0)
                # single model-time stamp delays stream 1's first attention by
                # ~half a step; real deps + priorities maintain the stagger
                with tc.tile_wait_until(STEP_MS / 2):
                    p1a(x, 1, 0)
                    p1b(x, 1, 0)
                for s in range(steps):
                    p2(x, 0, s)
                    if s + 1 < steps:
                        p1a(x, 0, s + 1)
                        p1b(x, 0, s + 1)
                    p2(x, 1, s)
                    probs_f(x, s)
                    if s + 1 < steps:
                        gates_hof(x, s + 1)
                        p1a(x, 1, s + 1)
                        p1b(x, 1, s + 1)
    if not nc.is_finalized():
        nc.finalize()
    return nc


def gates_hof(x, s):
    """Fused h-recurrent + onehot gate matmuls for BOTH streams (N=64).
    pg col layout [gi(4), k(4), b(64)] = [i | f | o | g], banks A=(i,f) B=(o,g)."""
    nc = x.nc
    pg = x.ps.tile([128, 1024], F32, tag="pg", name=f"pg{s}", bufs=2)
    x.pg = pg
    ohsl = x.ohT[:, s * BCORE : (s + 1) * BCORE]
    started = set()
    for gi, g in enumerate(GATE_ORDER):
        for k in range(HK):
            m = 4 * g + k
            col = pg[:, gi * 256 + k * BCORE : gi * 256 + (k + 1) * BCORE]
            bank = gi // 2
            for kk in range(HK):
                nc.tensor.matmul(
                    col,
                    x.w_hh[:, kk * G4 + 128 * m : kk * G4 + 128 * m + 128],
                    x.hTf[:, kk * BCORE : (kk + 1) * BCORE],
                    start=(bank not in started),
                    stop=False,
                    skip_group_check=True,
                )
                started.add(bank)
            nc.tensor.matmul(
                col, x.w_oh[:, 128 * m : 128 * m + 128], ohsl,
                start=False, stop=False, skip_group_check=True,
            )


def probs_f(x, s):
    """Fused output projection for both streams: [64, C] psum -> DRAM."""
    nc = x.nc
    pp_ = x.sc[0][0:BCORE, PP0:PP1]
    for k in range(HK):
        nc.tensor.matmul(
            pp_,
            x.hTf[:, k * BCORE : (k + 1) * BCORE],
            x.w_gen[:, k * C : (k + 1) * C],
            start=(k == 0),
            stop=False,
            skip_group_check=True,
        )
    nc.tensor.matmul(
        pp_, x.ones[0:1, :], x.b_gen[:],
        start=False, stop=True, skip_group_check=True,
    )
    po = x.psm.tile([BCORE, C], F32, tag="po", name="po", bufs=2)
    nc.vector.tensor_copy(po[:], pp_)
    nc.sync.dma_start(x.d_out[:, s, :], po[:])


def p1a(x, c, s):
    """Attention front half (DVE+ACT): hp broadcast, add, tanh -> th tiles."""
    nc = x.nc
    if s > 0:
        hp2 = x.psm.tile([128, HK * BC * 2], FP16, tag=f"hp2_{c}", name=f"hp2_{c}",
                         bufs=2)
        nc.vector.tensor_copy(
            hp2[:].rearrange("p (k b two) -> p k b two", k=HK, two=2),
            x.sc[c][:, PHP0:PHP1]
            .rearrange("p (k b) -> p k b", k=HK)
            .unsqueeze(3)
            .broadcast_to([128, HK, BC, 2]),
        )
        hp2v = hp2[:].rearrange("p (k b two) -> p k b two", k=HK, two=2)
    for kp in range(2):
        th = x.pw.tile([128, 2 * BT], TH_DT, tag=f"th{c}", name=f"th{c}_{kp}", bufs=2)
        x.th[c][kp] = th
        if s == 0:
            nc.scalar.activation(
                th[:], x.hproj[c][:, kp * 2 * BT : (kp + 1) * 2 * BT], AF.Tanh
            )
        else:
            a = x.pw.tile([128, 2 * BT], FP16, tag=f"a{c}", name=f"a{c}_{kp}", bufs=2)
            for kk in range(2):
                k = 2 * kp + kk
                nc.vector.tensor_add(
                    a[:, kk * BT : (kk + 1) * BT].rearrange(
                        "p (b t2 two) -> p b t2 two", b=BC, two=2
                    ),
                    x.hproj[c][:, k * BT : (k + 1) * BT].rearrange(
                        "p (b t2 two) -> p b t2 two", b=BC, two=2
                    ),
                    hp2v[:, k].unsqueeze(2).broadcast_to([128, BC, T // 2, 2]),
                )
            nc.scalar.activation(th[:], a[:], AF.Tanh)


def p1b(x, c, s):
    """Attention e-scores (PE): pe2[:, j] = sum_k th_k[:,128j:].T @ w_sc[:,k]."""
    nc = x.nc
    pe2 = x.sc[c][:, PE20:PE21]
    for kp in range(2):
        th = x.th[c][kp]
        for kk in range(2):
            k = 2 * kp + kk
            for j in range(NJ):
                nc.tensor.matmul(
                    pe2[:, j : j + 1],
                    th[:, kk * BT + 128 * j : kk * BT + 128 * j + 128],
                    x.w_sc[:, k : k + 1],
                    start=(k == 0 and j == 0),
                    stop=(k == HK - 1 and j == NJ - 1),
                    skip_group_check=True,
                )


def p2(x, c, s):
    """Stream tail: softmax, ctxT, ctx-gates, LSTM, php."""
    nc = x.nc
    steps = x.steps

    # -- softmax over t (transpose dance; no max-sub: |e| small) --
    e2 = x.psm.tile([128, NJ], F32, tag=f"e2_{c}", name=f"e2_{c}", bufs=2)
    nc.vector.tensor_copy(e2[:], x.sc[c][:, PE20:PE21])
    ptr = x.sc[c][0:NJ, PTR0:PTR1]
    nc.tensor.transpose(ptr, e2[:], x.id_f[:])
    ex = x.psm.tile([NJ, 128], FP16, tag=f"ex_{c}", name=f"ex_{c}", bufs=2)
    nc.scalar.activation(ex[:], ptr, AF.Exp, scale=1.0 / WS_SC)
    ssum = x.psm.tile([NJ, 2], F32, tag=f"ssum_{c}", name=f"ssum_{c}", bufs=2)
    nc.vector.reduce_sum(
        ssum[:], ex[:].rearrange("p (b t) -> p b t", b=2), axis=mybir.AxisListType.X
    )
    rinv = x.psm.tile([NJ, 2], F32, tag=f"rinv_{c}", name=f"rinv_{c}", bufs=2)
    nc.vector.reciprocal(rinv[:], ssum[:])
    al = x.psm.tile([NJ, 128], F32, tag=f"al_{c}", name=f"al_{c}", bufs=2)
    nc.vector.tensor_mul(
        al[:].rearrange("p (b t) -> p b t", b=2),
        ex[:].rearrange("p (b t) -> p b t", b=2),
        rinv[:].unsqueeze(2).broadcast_to([NJ, 2, T]),
    )
    pac = x.sc[c][:, PAC0:PAC1]
    nc.tensor.transpose(pac, al[:], x.id_f[0:NJ, 0:NJ])
    adv = x.ad[c][:].rearrange("p (i two) -> p i two", two=2)
    for jj in range(2):
        nc.vector.tensor_copy(
            adv[64 * jj : 64 * jj + 64, :, jj], pac[64 * jj : 64 * jj + 64, :]
        )

    # -- ctxT[d, b] direct: lhsT = enc tile (d -> partitions), rhs = ad pair --
    pctxT = x.sc[c][:, PCTX0:PCTX1]
    for m in range(HK):
        for i in range(BC // 2):
            nc.tensor.matmul(
                pctxT[:, m * BC + 2 * i : m * BC + 2 * i + 2],
                x.enc_sb[c][:, 512 * i + 128 * m : 512 * i + 128 * m + 128],
                x.ad[c][:, 2 * i : 2 * i + 2],
                start=True,
                stop=True,
            )
    nc.vector.tensor_scalar_mul(x.ctxT[c][:], pctxT, 1.0 / ENC_SC)

    # -- ctx contribution to gates (per-stream half, N=32) --
    pg = x.pg
    for gi, g in enumerate(GATE_ORDER):
        for k in range(HK):
            m = 4 * g + k
            col = pg[:, gi * 256 + k * BCORE + c * BC : gi * 256 + k * BCORE + (c + 1) * BC]
            for kk in range(HK):
                nc.tensor.matmul(
                    col,
                    x.w_ctx[:, kk * G4 + 128 * m : kk * G4 + 128 * m + 128],
                    x.ctxT[c][:, kk * BC : (kk + 1) * BC],
                    start=False,
                    stop=(c == 1 and gi % 2 == 1 and k == HK - 1 and kk == HK - 1),
                    skip_group_check=True,
                )

    # -- LSTM (h' = 2h trick: sigma(x) = 0.5*(tanh(x/2)+1)) --
    pgv = pg[:].rearrange("p (gi k b) -> p gi k b", gi=4, k=HK)
    tifo = x.psm.tile([128, 3 * HK * BC], FP16, tag=f"tifo_{c}", name=f"tifo_{c}",
                      bufs=2)
    nc.scalar.activation(
        tifo[:].rearrange("p (gi k b) -> p gi k b", gi=3, k=HK),
        pgv[:, 0:3, :, c * BC : (c + 1) * BC],
        AF.Tanh, scale=0.5 / WG_SC,
    )
    tg = x.psm.tile([128, HK * BC], FP16, tag=f"tg_{c}", name=f"tg_{c}", bufs=2)
    nc.scalar.activation(
        tg[:].rearrange("p (k b) -> p k b", k=HK),
        pgv[:, 3, :, c * BC : (c + 1) * BC],
        AF.Tanh, scale=1.0 / WG_SC,
    )
    # C' = 2c: C'_new = 0.5*(tanh_f+1)*C'_old + (tanh_i+1)*tanh_g
    m2 = x.psm.tile([128, HK * BC], F32, tag=f"m2_{c}", name=f"m2_{c}", bufs=2)
    nc.vector.scalar_tensor_tensor(
        m2[:], tifo[:, 0 : HK * BC], 1.0, tg[:], ALU.add, ALU.mult
    )
    if s == 0:
        nc.vector.tensor_copy(x.cT[c][:], m2[:])
    else:
        m1 = x.psm.tile([128, HK * BC], F32, tag=f"m1_{c}", name=f"m1_{c}", bufs=2)
        nc.vector.scalar_tensor_tensor(
            m1[:], tifo[:, HK * BC : 2 * HK * BC], 1.0, x.cT[c][:], ALU.add, ALU.mult
        )
        nc.vector.scalar_tensor_tensor(
            x.cT[c][:], m1[:], 0.5, m2[:], ALU.mult, ALU.add
        )
    tc_ = x.psm.tile([128, HK * BC], FP16, tag=f"tc_{c}", name=f"tc_{c}", bufs=2)
    nc.scalar.activation(tc_[:], x.cT[c][:], AF.Tanh, scale=0.5)
    # H' = 2h = (tanh_o + 1) * tanh(c), into this stream's half of fused hT
    nc.vector.scalar_tensor_tensor(
        x.hTf[:].rearrange("p (k b) -> p k b", k=HK)[:, :, c * BC : (c + 1) * BC],
        tifo[:].rearrange("p (gi k b) -> p gi k b", gi=3, k=HK)[:, 2],
        1.0,
        tc_[:].rearrange("p (k b) -> p k b", k=HK),
        ALU.add, ALU.mult,
    )

    # -- php = w_h2h' @ H' (next step's attention shift) --
    if s < steps - 1:
        php = x.sc[c][:, PHP0:PHP1]
        for m in range(HK):
            for k in range(HK):
                nc.tensor.matmul(
                    php[:, m * BC : (m + 1) * BC],
                    x.w_h2h[:, k * H + 128 * m : k * H + 128 * m + 128],
                    x.hTf[:, k * BCORE + c * BC : k * BCORE + (c + 1) * BC],
                    start=(m == 0 and k == 0),
                    stop=(m == HK - 1 and k == HK - 1),
                    skip_group_check=True,
                )


# ------------------------- host side -------------------------


def prep_inputs(encoder_output, text, w_i2h, w_h2h, b_h2h, w_score, w_ih, w_hh,
                b_ih, b_hh, w_gen, b_gen, steps=S):
    """Build per-core input maps (numpy only)."""
    enc = np.asarray(encoder_output, np.float32)
    text = np.asarray(text)

    wid = {}
    wid["w_i2hT"] = _tile128(np.asarray(w_i2h, np.float32).T, F16)
    # w_h2h' = 0.5*w_h2h: php(H') = w_h2h @ h exactly
    wid["w_h2hT"] = _tile128(0.5 * np.asarray(w_h2h, np.float32).T, F16)
    wid["w_scoreT"] = _tile128(
        WS_SC * np.asarray(w_score, np.float32).reshape(H, 1), TH_NP
    )
    # gate weights: ctx part xWG, h part xWG/2 (h'=2h), oh part + bias xWG
    wid["w_ctxT"] = _tile128(WG_SC * np.asarray(w_ih, np.float32)[:, :D].T, W_NP)
    wid["w_hhT"] = _tile128(0.5 * WG_SC * np.asarray(w_hh, np.float32).T, F16)
    woh = np.zeros((128, G4), np.float32)
    woh[:C] = WG_SC * np.asarray(w_ih, np.float32)[:, D:].T
    woh[C] = WG_SC * (np.asarray(b_ih, np.float32) + np.asarray(b_hh, np.float32))
    wid["w_ohT"] = woh.astype(F16)
    wid["w_genT"] = _tile128(0.5 * np.asarray(w_gen, np.float32).T, F16)
    wid["b_gen"] = np.asarray(b_gen, np.float32).reshape(1, C).astype(F16)
    wid["b_h2hT"] = np.ascontiguousarray(
        np.asarray(b_h2h, np.float32).reshape(HK, 128).T
    )
    wid["id_f"] = np.eye(128, dtype=np.float32)
    wid["ones_row"] = np.ones((1, BCORE), F16)

    in_maps = []
    for core in range(NCORES):
        rows = slice(core * BCORE, (core + 1) * BCORE)
        ec = enc[rows]  # [64, T, D]
        enc_sb = np.zeros((NCHUNK, 128, NJ * 512), ENC_NP)
        encT_sb = np.zeros((NCHUNK, 128, HK * BT), F16)
        for c in range(NCHUNK):
            flat = ec[c * BC : (c + 1) * BC].reshape(BT, D)  # b-major (b,t) rows
            enc_sb[c] = _tile128(np.clip(ENC_SC * flat, -15.5, 15.5), ENC_NP)
            encT_sb[c] = _tile128(np.ascontiguousarray(flat.T), F16)
        oh = np.zeros((128, steps * BCORE), F16)
        tx = text[rows]  # [64, S]
        for s in range(steps):
            oh[tx[:, s].astype(np.int64), s * BCORE + np.arange(BCORE)] = 1.0
        oh[C] = 1.0
        m = dict(wid)
        m["enc_sb"] = enc_sb
        m["encT_sb"] = encT_sb
        m["ohT_sb"] = oh
        in_maps.append(m)
    return in_maps


_NC_CACHE = {}


def get_nc(steps=S):
    if steps not in _NC_CACHE:
        _NC_CACHE[steps] = build_nc(steps)
    return _NC_CACHE[steps]


def run(inputs, steps=S, trace=False):
    nc = get_nc(steps)
    in_maps = prep_inputs(**inputs, steps=steps)
    res = run_bass_kernel_spmd(nc, in_maps, list(range(NCORES)), trace=trace)
    out = np.concatenate([res.results[i]["probs"] for i in range(NCORES)], axis=0)
    return out.astype(np.float32), res


def kernel(**inputs):
    out, _ = run(inputs)
    return out


# revision 21
# speedup vs baseline: 1.2078x; 1.0693x over previous
"""Trainium2 Bass kernel: attention-LSTM decoder (nn_Attention_74698071212133).

Sharding: data-parallel over batch across 8 NeuronCores (64 rows each), weights
replicated.  Each core runs TWO mostly-independent 32-row recurrence streams
(chunks), phase-staggered half a step apart (tile_wait_until stamps steer the
Tile scheduler) so one stream's serial tail (softmax/ctx/gates/LSTM) hides
under the other stream's attention tanh chain (ScalarE tanh is the hard
per-step floor: B*T*H/8 elems / 128 lanes / 1.2GHz ~ 13.7us/step).

The h-recurrent + onehot gate matmuls and the output projection are fused
across the two streams (N=64, shared weight loads, 16-bit weights); the
ctx-dependent gate half stays per-stream (N=32, fp8 weights) to preserve the
stagger.  Precision: fp16 for all 16-bit tensors; fp8e3 only on the
LDWEIGHTS-bound paths (tanh scores, enc for ctxT, w_ctx), with scales folded
into activation scale parameters; sigmoid via 0.5*(tanh(x/2)+1) with h'=2h so
the 0.5s fold into host-scaled weights; scalar_tensor_tensor fuses (x+1)*y.
"""

import sys

sys.path.insert(0, "/opt/trn_rl_repo")

import numpy as np
import ml_dtypes

import concourse.bass as bass
import concourse.mybir as mybir
import concourse.tile as tile
from concourse import bacc
from concourse.bass_utils import run_bass_kernel_spmd

F16 = np.float16
F8 = ml_dtypes.float8_e3m4
F32 = mybir.dt.float32
FP16 = mybir.dt.float16
FP8 = mybir.dt.float8e3
AF = mybir.ActivationFunctionType
ALU = mybir.AluOpType

# Problem constants
B, T, D, H, C, S = 512, 64, 512, 512, 38, 26
NCORES = 8
BCORE = B // NCORES  # 64
NCHUNK = 2
G4 = 4 * H  # 2048
HK = H // 128  # 4 h-tiles
BC = BCORE // NCHUNK  # 32 batch per stream
BT = BC * T  # 2048 flattened (b, t) per stream
NJ = BT // 128  # 16

# fp8 path toggles
FP8_TH = True   # tanh output + w_score in fp8e3 (e-score LDW fast)
FP8_ENC = True  # enc for ctxT matmul in fp8e3
FP8_W = True    # ctx-gate weights in fp8e3 (x32 scaled)
WS_SC = 32.0 if FP8_TH else 1.0   # w_score host scale, folded out in exp
WG_SC = 32.0 if FP8_W else 1.0    # gate-weight host scale, folded out in ACT
ENC_SC = 4.0 if FP8_ENC else 1.0  # enc host scale, folded out in ctxT copy

TH_DT = FP8 if FP8_TH else FP16
ENC_DT = FP8 if FP8_ENC else FP16
W_DT = FP8 if FP8_W else FP16
TH_NP = F8 if FP8_TH else F16
ENC_NP = F8 if FP8_ENC else F16
W_NP = F8 if FP8_W else F16

# scheduler phase stagger (ms of model time per step)
STEP_MS = 0.021

# gate source order producing psum col layout [i | f | o | g]
GATE_ORDER = (0, 1, 3, 2)

# per-stream psum scratch-bank column layout (f32 cols of a [128, 512] tile)
PE20, PE21 = 0, NJ                    # e-scores [128, 16]
PTR0, PTR1 = 16, 144                  # e transposed [16, 128]
PAC0, PAC1 = 144, 160                 # alpha transposed back [128, 16]
PCTX0, PCTX1 = 160, 288               # ctxT [128, 128]
PP0, PP1 = 288, 326                   # probs [64, 38] (stream-0 bank only)
PHP0, PHP1 = 326, 454                 # hp for next step [128, 128]


def _tile128(a, dt):
    """[R, N] with R = r*128 -> [128, r*N] col-block layout (block k = rows 128k..)."""
    r = a.shape[0] // 128
    return np.ascontiguousarray(
        a.reshape(r, 128, a.shape[1]).transpose(1, 0, 2).reshape(128, -1)
    ).astype(dt)


class Ctx:
    """Per-build handles."""


def build_nc(steps=S):
    nc = bacc.Bacc()
    dp = nc.declare_dram_parameter
    x = Ctx()
    x.nc = nc
    x.steps = steps

    d_enc = dp("enc_sb", [NCHUNK, 128, NJ * 512], ENC_DT, isOutput=False)
    d_encT = dp("encT_sb", [NCHUNK, 128, HK * BT], FP16, isOutput=False)
    d_oh = dp("ohT_sb", [128, steps * BCORE], FP16, isOutput=False)
    d_wi2h = dp("w_i2hT", [128, HK * H], FP16, isOutput=False)
    d_wh2h = dp("w_h2hT", [128, HK * H], FP16, isOutput=False)
    d_wsc = dp("w_scoreT", [128, HK], TH_DT, isOutput=False)
    d_wctx = dp("w_ctxT", [128, HK * G4], W_DT, isOutput=False)
    d_whh = dp("w_hhT", [128, HK * G4], FP16, isOutput=False)
    d_woh = dp("w_ohT", [128, G4], FP16, isOutput=False)
    d_wgen = dp("w_genT", [128, HK * C], FP16, isOutput=False)
    d_bgen = dp("b_gen", [1, C], FP16, isOutput=False)
    d_bh2h = dp("b_h2hT", [128, HK], F32, isOutput=False)
    d_idf = dp("id_f", [128, 128], F32, isOutput=False)
    d_ones = dp("ones_row", [1, BCORE], FP16, isOutput=False)
    d_out = dp("probs", [BCORE, steps, C], F32, isOutput=True)

    with tile.TileContext(nc) as tc:
        with (
            tc.tile_pool(name="consts", bufs=1) as pc,
            tc.tile_pool(name="persist", bufs=1) as pp,
        ):
            def cload(dram, shape, dt):
                t_ = pc.tile(list(shape), dt, name=dram.tensor.name + "_sb")
                nc.sync.dma_start(t_[:], dram)
                return t_

            x.w_h2h = cload(d_wh2h[:], [128, HK * H], FP16)
            x.w_sc = cload(d_wsc[:], [128, HK], TH_DT)
            x.w_ctx = cload(d_wctx[:], [128, HK * G4], W_DT)
            x.w_hh = cload(d_whh[:], [128, HK * G4], FP16)
            x.w_oh = cload(d_woh[:], [128, G4], FP16)
            x.w_gen = cload(d_wgen[:], [128, HK * C], FP16)
            x.b_gen = cload(d_bgen[:], [1, C], FP16)
            x.id_f = cload(d_idf[:], [128, 128], F32)
            x.ones = cload(d_ones[:], [1, BCORE], FP16)
            x.ohT = cload(d_oh[:], [128, steps * BCORE], FP16)
            w_i2h = cload(d_wi2h[:], [128, HK * H], FP16)
            b_h2h = cload(d_bh2h[:], [128, HK], F32)

            # ---- persistent state: hT fused across streams, rest per-stream ----
            x.hTf = pp.tile([128, HK * BCORE], FP16, tag="hTf", name="hTf")
            nc.vector.memset(x.hTf[:], 0.0)
            x.cT, x.ctxT, x.ad, x.enc_sb, x.hproj = [], [], [], [], []
            for c in range(NCHUNK):
                c_ = pp.tile([128, HK * BC], F32, tag=f"cT{c}", name=f"cT{c}")
                ct = pp.tile([128, HK * BC], FP16, tag=f"ctxT{c}", name=f"ctxT{c}")
                a_ = pp.tile([128, 2 * NJ], FP16, tag=f"ad{c}", name=f"ad{c}")
                nc.vector.memset(a_[:], 0.0)
                x.cT.append(c_)
                x.ctxT.append(ct)
                x.ad.append(a_)
                e_ = pp.tile([128, NJ * 512], ENC_DT, tag=f"enc{c}", name=f"enc{c}")
                for q in range(4):
                    w = NJ * 512 // 4
                    nc.sync.dma_start(
                        e_[:, q * w : (q + 1) * w], d_enc[c, :, q * w : (q + 1) * w]
                    )
                x.enc_sb.append(e_)
                x.hproj.append(
                    pp.tile([128, HK * BT], FP16, tag=f"hproj{c}", name=f"hproj{c}")
                )

            # ---- init: H_projT = w_i2h @ encT + b_h2h ----
            with (
                tc.tile_pool(name="encT", bufs=1) as pet,
                tc.tile_pool(name="initps", bufs=4, space="PSUM") as pips,
            ):
                for c in range(NCHUNK):
                    et = pet.tile([128, HK * BT], FP16, tag=f"encT{c}", name=f"encT{c}")
                    for q in range(4):
                        w = HK * BT // 4
                        nc.sync.dma_start(
                            et[:, q * w : (q + 1) * w],
                            d_encT[c, :, q * w : (q + 1) * w],
                        )
                    for m in range(HK):
                        for n in range(BT // 512):
                            ps = pips.tile([128, 512], F32, tag="initp")
                            for k in range(HK):
                                nc.tensor.matmul(
                                    ps[:],
                                    w_i2h[:, k * H + 128 * m : k * H + 128 * m + 128],
                                    et[:, k * BT + 512 * n : k * BT + 512 * n + 512],
                                    start=(k == 0),
                                    stop=(k == HK - 1),
                                )
                            nc.scalar.activation(
                                x.hproj[c][:, m * BT + 512 * n : m * BT + 512 * n + 512],
                                ps[:],
                                AF.Identity,
                                bias=b_h2h[:, m : m + 1],
                            )

            # ---- decode: phase-staggered two-stream pipeline ----
            with (
                tc.tile_pool(name="work", bufs=1) as pw,
                tc.tile_pool(name="small", bufs=1) as psm,
                tc.tile_pool(name="ps", bufs=1, space="PSUM") as ps,
            ):
                x.pw, x.psm, x.ps = pw, psm, ps
                x.d_out = d_out
                x.pg = None
                x.sc = [
                    ps.tile([128, 512], F32, tag=f"sc{c}", name=f"sc{c}", bufs=1)
                    for c in range(NCHUNK)
                ]
                x.th = [[None, None], [None, None]]
                x.a = [None, None]
                x.tifo = [None, None]

                F = tc.no_sync_barrier
                gates_hof(x, 0)
                for k in range(HK):
                    tanh_k(x, 0, 0, k)
                    escore_k(x, 0, 0, k)
                    F()
                # steady state: stream 1 runs half a step behind stream 0;
                # tanh quarters of one stream fill the other stream's tail
                # gaps on ACT.  no_sync fences pin the per-engine order.
                for s in range(steps):
                    last = s + 1 >= steps
                    tanh_k(x, 1, s, 0); escore_k(x, 1, s, 0); F()
                    p2a(x, 0, s); F()
                    tanh_k(x, 1, s, 1); escore_k(x, 1, s, 1); F()
                    tanh_k(x, 1, s, 2); escore_k(x, 1, s, 2); F()
                    p2b1(x, 0, s); F()
                    p2b2(x, 0, s); F()
                    tanh_k(x, 1, s, 3); escore_k(x, 1, s, 3); F()
                    if not last:
                        tanh_k(x, 0, s + 1, 0); escore_k(x, 0, s + 1, 0); F()
                    p2a(x, 1, s); F()
                    if not last:
                        tanh_k(x, 0, s + 1, 1); escore_k(x, 0, s + 1, 1); F()
                        tanh_k(x, 0, s + 1, 2); escore_k(x, 0, s + 1, 2); F()
                    p2b1(x, 1, s); F()
                    p2b2(x, 1, s)
                    probs_f(x, s)
                    if not last:
                        gates_hof(x, s + 1)
                    F()
                    if not last:
                        tanh_k(x, 0, s + 1, 3); escore_k(x, 0, s + 1, 3); F()
    if not nc.is_finalized():
        nc.finalize()
    return nc


def gates_hof(x, s):
    """Fused h-recurrent + onehot gate matmuls for BOTH streams (N=64).
    pg col layout [gi(4), k(4), b(64)] = [i | f | o | g], banks A=(i,f) B=(o,g)."""
    nc = x.nc
    pg = x.ps.tile([128, 1024], F32, tag="pg", name=f"pg{s}", bufs=2)
    x.pg = pg
    ohsl = x.ohT[:, s * BCORE : (s + 1) * BCORE]
    started = set()
    for gi, g in enumerate(GATE_ORDER):
        for k in range(HK):
            m = 4 * g + k
            col = pg[:, gi * 256 + k * BCORE : gi * 256 + (k + 1) * BCORE]
            bank = gi // 2
            for kk in range(HK):
                nc.tensor.matmul(
                    col,
                    x.w_hh[:, kk * G4 + 128 * m : kk * G4 + 128 * m + 128],
                    x.hTf[:, kk * BCORE : (kk + 1) * BCORE],
                    start=(bank not in started),
                    stop=False,
                    skip_group_check=True,
                )
                started.add(bank)
            nc.tensor.matmul(
                col, x.w_oh[:, 128 * m : 128 * m + 128], ohsl,
                start=False, stop=False, skip_group_check=True,
            )


def probs_f(x, s):
    """Fused output projection for both streams: [64, C] psum -> DRAM."""
    nc = x.nc
    pp_ = x.sc[1][0:BCORE, PP0:PP1]
    for k in range(HK):
        nc.tensor.matmul(
            pp_,
            x.hTf[:, k * BCORE : (k + 1) * BCORE],
            x.w_gen[:, k * C : (k + 1) * C],
            start=(k == 0),
            stop=False,
            skip_group_check=True,
        )
    nc.tensor.matmul(
        pp_, x.ones[0:1, :], x.b_gen[:],
        start=False, stop=True, skip_group_check=True,
    )
    po = x.psm.tile([BCORE, C], F32, tag="po", name="po", bufs=2)
    nc.vector.tensor_copy(po[:], pp_)
    nc.sync.dma_start(x.d_out[:, s, :], po[:])


def hp2_adds(x, c, s):
    """hp broadcast + the 4 per-k adds feeding step s's tanh (DVE)."""
    nc = x.nc
    hp2 = x.psm.tile([128, HK * BC * 2], FP16, tag=f"hp2_{c}", name=f"hp2_{c}",
                     bufs=2)
    nc.vector.tensor_copy(
        hp2[:].rearrange("p (k b two) -> p k b two", k=HK, two=2),
        x.sc[c][:, PHP0:PHP1]
        .rearrange("p (k b) -> p k b", k=HK)
        .unsqueeze(3)
        .broadcast_to([128, HK, BC, 2]),
    )
    hp2v = hp2[:].rearrange("p (k b two) -> p k b two", k=HK, two=2)
    x.a[c] = []
    for kp in range(2):
        a = x.pw.tile([128, 2 * BT], FP16, tag=f"a{c}", name=f"a{c}_{kp}", bufs=2)
        x.a[c].append(a)
        for kk in range(2):
            k = 2 * kp + kk
            nc.vector.tensor_add(
                a[:, kk * BT : (kk + 1) * BT].rearrange(
                    "p (b t2 two) -> p b t2 two", b=BC, two=2
                ),
                x.hproj[c][:, k * BT : (k + 1) * BT].rearrange(
                    "p (b t2 two) -> p b t2 two", b=BC, two=2
                ),
                hp2v[:, k].unsqueeze(2).broadcast_to([128, BC, T // 2, 2]),
            )


def tanh_k(x, c, s, k):
    """One k-tile of the attention tanh (ACT, [128, 2048])."""
    nc = x.nc
    kp, kk = k // 2, k % 2
    if kk == 0:
        x.th[c][kp] = x.pw.tile([128, 2 * BT], TH_DT, tag=f"th{c}",
                                name=f"th{c}_{kp}", bufs=2)
    th = x.th[c][kp]
    if s == 0:
        src_ = x.hproj[c][:, k * BT : (k + 1) * BT]
    else:
        src_ = x.a[c][kp][:, kk * BT : (kk + 1) * BT]
    nc.scalar.activation(th[:, kk * BT : (kk + 1) * BT], src_, AF.Tanh)


def escore_k(x, c, s, k):
    """e-score contribution of one k-tile (PE): pe2[:, j] += th_k.T @ w_sc[k]."""
    nc = x.nc
    kp, kk = k // 2, k % 2
    th = x.th[c][kp]
    pe2 = x.sc[c][:, PE20:PE21]
    for j in range(NJ):
        nc.tensor.matmul(
            pe2[:, j : j + 1],
            th[:, kk * BT + 128 * j : kk * BT + 128 * j + 128],
            x.w_sc[:, k : k + 1],
            start=(k == 0 and j == 0),
            stop=(k == HK - 1 and j == NJ - 1),
            skip_group_check=True,
        )


def p2a(x, c, s):
    """Softmax over t + ctxT (DVE/PE/ACT)."""
    nc = x.nc
    e2 = x.psm.tile([128, NJ], F32, tag=f"e2_{c}", name=f"e2_{c}", bufs=2)
    nc.vector.tensor_copy(e2[:], x.sc[c][:, PE20:PE21])
    ptr = x.sc[c][0:NJ, PTR0:PTR1]
    nc.tensor.transpose(ptr, e2[:], x.id_f[:])
    ex = x.psm.tile([NJ, 128], FP16, tag=f"ex_{c}", name=f"ex_{c}", bufs=2)
    nc.scalar.activation(ex[:], ptr, AF.Exp, scale=1.0 / WS_SC)
    ssum = x.psm.tile([NJ, 2], F32, tag=f"ssum_{c}", name=f"ssum_{c}", bufs=2)
    nc.vector.reduce_sum(
        ssum[:], ex[:].rearrange("p (b t) -> p b t", b=2), axis=mybir.AxisListType.X
    )
    rinv = x.psm.tile([NJ, 2], F32, tag=f"rinv_{c}", name=f"rinv_{c}", bufs=2)
    nc.vector.reciprocal(rinv[:], ssum[:])
    al = x.psm.tile([NJ, 128], F32, tag=f"al_{c}", name=f"al_{c}", bufs=2)
    nc.vector.tensor_mul(
        al[:].rearrange("p (b t) -> p b t", b=2),
        ex[:].rearrange("p (b t) -> p b t", b=2),
        rinv[:].unsqueeze(2).broadcast_to([NJ, 2, T]),
    )
    pac = x.sc[c][:, PAC0:PAC1]
    nc.tensor.transpose(pac, al[:], x.id_f[0:NJ, 0:NJ])
    adv = x.ad[c][:].rearrange("p (i two) -> p i two", two=2)
    for jj in range(2):
        nc.vector.tensor_copy(
            adv[64 * jj : 64 * jj + 64, :, jj], pac[64 * jj : 64 * jj + 64, :]
        )
    pctxT = x.sc[c][:, PCTX0:PCTX1]
    for m in range(HK):
        for i in range(BC // 2):
            nc.tensor.matmul(
                pctxT[:, m * BC + 2 * i : m * BC + 2 * i + 2],
                x.enc_sb[c][:, 512 * i + 128 * m : 512 * i + 128 * m + 128],
                x.ad[c][:, 2 * i : 2 * i + 2],
                start=True,
                stop=True,
            )
    nc.vector.tensor_scalar_mul(x.ctxT[c][:], pctxT, 1.0 / ENC_SC)


def p2b1(x, c, s):
    """ctx half of the gates (PE, fp8) + the ifo tanh (ACT)."""
    nc = x.nc
    pg = x.pg
    for gi, g in enumerate(GATE_ORDER):
        for k in range(HK):
            m = 4 * g + k
            col = pg[:, gi * 256 + k * BCORE + c * BC : gi * 256 + k * BCORE + (c + 1) * BC]
            for kk in range(HK):
                nc.tensor.matmul(
                    col,
                    x.w_ctx[:, kk * G4 + 128 * m : kk * G4 + 128 * m + 128],
                    x.ctxT[c][:, kk * BC : (kk + 1) * BC],
                    start=False,
                    stop=(c == 1 and gi % 2 == 1 and k == HK - 1 and kk == HK - 1),
                    skip_group_check=True,
                )
    pgv = pg[:].rearrange("p (gi k b) -> p gi k b", gi=4, k=HK)
    tifo = x.psm.tile([128, 3 * HK * BC], FP16, tag=f"tifo_{c}", name=f"tifo_{c}",
                      bufs=2)
    nc.scalar.activation(
        tifo[:].rearrange("p (gi k b) -> p gi k b", gi=3, k=HK),
        pgv[:, 0:3, :, c * BC : (c + 1) * BC],
        AF.Tanh, scale=0.5 / WG_SC,
    )
    x.tifo[c] = tifo


def p2b2(x, c, s):
    """LSTM tail: tg, cell update, h' write, php, next-step adds."""
    nc = x.nc
    steps = x.steps
    pg = x.pg
    pgv = pg[:].rearrange("p (gi k b) -> p gi k b", gi=4, k=HK)
    tifo = x.tifo[c]
    tg = x.psm.tile([128, HK * BC], FP16, tag=f"tg_{c}", name=f"tg_{c}", bufs=2)
    nc.scalar.activation(
        tg[:].rearrange("p (k b) -> p k b", k=HK),
        pgv[:, 3, :, c * BC : (c + 1) * BC],
        AF.Tanh, scale=1.0 / WG_SC,
    )
    # C' = 2c: C'_new = 0.5*(tanh_f+1)*C'_old + (tanh_i+1)*tanh_g
    m2 = x.psm.tile([128, HK * BC], F32, tag=f"m2_{c}", name=f"m2_{c}", bufs=2)
    nc.vector.scalar_tensor_tensor(
        m2[:], tifo[:, 0 : HK * BC], 1.0, tg[:], ALU.add, ALU.mult
    )
    if s == 0:
        nc.vector.tensor_copy(x.cT[c][:], m2[:])
    else:
        m1 = x.psm.tile([128, HK * BC], F32, tag=f"m1_{c}", name=f"m1_{c}", bufs=2)
        nc.vector.scalar_tensor_tensor(
            m1[:], tifo[:, HK * BC : 2 * HK * BC], 1.0, x.cT[c][:], ALU.add, ALU.mult
        )
        nc.vector.scalar_tensor_tensor(
            x.cT[c][:], m1[:], 0.5, m2[:], ALU.mult, ALU.add
        )
    tc_ = x.psm.tile([128, HK * BC], FP16, tag=f"tc_{c}", name=f"tc_{c}", bufs=2)
    nc.scalar.activation(tc_[:], x.cT[c][:], AF.Tanh, scale=0.5)
    # H' = 2h = (tanh_o + 1) * tanh(c), into this stream's half of fused hT
    nc.vector.scalar_tensor_tensor(
        x.hTf[:].rearrange("p (k b) -> p k b", k=HK)[:, :, c * BC : (c + 1) * BC],
        tifo[:].rearrange("p (gi k b) -> p gi k b", gi=3, k=HK)[:, 2],
        1.0,
        tc_[:].rearrange("p (k b) -> p k b", k=HK),
        ALU.add, ALU.mult,
    )
    if s < steps - 1:
        php = x.sc[c][:, PHP0:PHP1]
        for m in range(HK):
            for k in range(HK):
                nc.tensor.matmul(
                    php[:, m * BC : (m + 1) * BC],
                    x.w_h2h[:, k * H + 128 * m : k * H + 128 * m + 128],
                    x.hTf[:, k * BCORE + c * BC : k * BCORE + (c + 1) * BC],
                    start=(m == 0 and k == 0),
                    stop=(m == HK - 1 and k == HK - 1),
                    skip_group_check=True,
                )
        hp2_adds(x, c, s + 1)


# ------------------------- host side -------------------------


def prep_inputs(encoder_output, text, w_i2h, w_h2h, b_h2h, w_score, w_ih, w_hh,
                b_ih, b_hh, w_gen, b_gen, steps=S):
    """Build per-core input maps (numpy only)."""
    enc = np.asarray(encoder_output, np.float32)
    text = np.asarray(text)

    wid = {}
    wid["w_i2hT"] = _tile128(np.asarray(w_i2h, np.float32).T, F16)
    # w_h2h' = 0.5*w_h2h: php(H') = w_h2h @ h exactly
    wid["w_h2hT"] = _tile128(0.5 * np.asarray(w_h2h, np.float32).T, F16)
    wid["w_scoreT"] = _tile128(
        WS_SC * np.asarray(w_score, np.float32).reshape(H, 1), TH_NP
    )
    # gate weights: ctx part xWG, h part xWG/2 (h'=2h), oh part + bias xWG
    wid["w_ctxT"] = _tile128(WG_SC * np.asarray(w_ih, np.float32)[:, :D].T, W_NP)
    wid["w_hhT"] = _tile128(0.5 * WG_SC * np.asarray(w_hh, np.float32).T, F16)
    woh = np.zeros((128, G4), np.float32)
    woh[:C] = WG_SC * np.asarray(w_ih, np.float32)[:, D:].T
    woh[C] = WG_SC * (np.asarray(b_ih, np.float32) + np.asarray(b_hh, np.float32))
    wid["w_ohT"] = woh.astype(F16)
    wid["w_genT"] = _tile128(0.5 * np.asarray(w_gen, np.float32).T, F16)
    wid["b_gen"] = np.asarray(b_gen, np.float32).reshape(1, C).astype(F16)
    wid["b_h2hT"] = np.ascontiguousarray(
        np.asarray(b_h2h, np.float32).reshape(HK, 128).T
    )
    wid["id_f"] = np.eye(128, dtype=np.float32)
    wid["ones_row"] = np.ones((1, BCORE), F16)

    in_maps = []
    for core in range(NCORES):
        rows = slice(core * BCORE, (core + 1) * BCORE)
        ec = enc[rows]  # [64, T, D]
        enc_sb = np.zeros((NCHUNK, 128, NJ * 512), ENC_NP)
        encT_sb = np.zeros((NCHUNK, 128, HK * BT), F16)
        for c in range(NCHUNK):
            flat = ec[c * BC : (c + 1) * BC].reshape(BT, D)  # b-major (b,t) rows
            enc_sb[c] = _tile128(np.clip(ENC_SC * flat, -15.5, 15.5), ENC_NP)
            encT_sb[c] = _tile128(np.ascontiguousarray(flat.T), F16)
        oh = np.zeros((128, steps * BCORE), F16)
        tx = text[rows]  # [64, S]
        for s in range(steps):
            oh[tx[:, s].astype(np.int64), s * BCORE + np.arange(BCORE)] = 1.0
        oh[C] = 1.0
        m = dict(wid)
        m["enc_sb"] = enc_sb
        m["encT_sb"] = encT_sb
        m["ohT_sb"] = oh
        in_maps.append(m)
    return in_maps


_NC_CACHE = {}


def get_nc(steps=S):
    if steps not in _NC_CACHE:
        _NC_CACHE[steps] = build_nc(steps)
    return _NC_CACHE[steps]


def run(inputs, steps=S, trace=False):
    nc = get_nc(steps)
    in_maps = prep_inputs(**inputs, steps=steps)
    res = run_bass_kernel_spmd(nc, in_maps, list(range(NCORES)), trace=trace)
    out = np.concatenate([res.results[i]["probs"] for i in range(NCORES)], axis=0)
    return out.astype(np.float32), res


def kernel(**inputs):
    out, _ = run(inputs)
    return out
